# revision 56
# baseline (speedup 1.0000x reference)
"""Multi-head attention (B=4, T=2048, D=768, H=12) on 8 NeuronCores.

Sharding: core c handles batch b = c//2 and head-group g = c%2 (heads
6g..6g+5).  Each core computes its 6 heads' attention and a partial
output projection; the host sums the two partials per batch and adds
the bias terms (v-bias folds through w_proj since softmax rows sum to 1).

Device formulation (cost-model-shaped: every matmul streams with full
128-wide output partitions; PE is kept continuously busy so the p-state
stays at 2.4 GHz):

  qT/kT [384, 2048] = W.T @ xT  (bf16 inputs, f32 psum, f32r qT/kT)
  S^T [kpos 128, q] = kT_h.T @ qT_h    per (head, kpos-tile)
  P^T = exp(S^T)  -- ACT engine, or offloaded to DVE via the
        exp-as-int-bitcast trick (P in fp16)
  O [q 128, 64] += P^T_chunk.T @ v_h   accumulated over kpos tiles
  den[q, 1]    += P^T_chunk.T @ ones
  O_sb = O * (1/den)  (DVE per-partition scalar), fp16
  oT = DMA-crossbar transpose of O_sb per (pair, q-tile)
  y = oT.T @ wp  partial, fp16, host adds partner core + bias row

Schedule: a flat software pipeline over 12 sweeps x 16 kpos-tiles
(ticks): scores(t+1) | exp(t) | AV(t-2), with QKV projections of later
head-pairs, the output projection, and V emission injected as PE filler
paced by deadlines.  A fraction of exp tiles per sweep runs on DVE so
ACT never gates PE.
"""

import numpy as np

EMBED = 768
HEADS = 12
HD = 64
SCALE = HD ** -0.5
B, T = 4, 2048
NCORES = 8
HPC = 6            # heads per core
DL = HPC * HD      # 384 local model dims per core

NDT = EMBED // 128   # 6 contraction tiles over embed dim
NKT = T // 128       # 16 key-position tiles
NQT = T // 128       # 16 query row tiles
CW = 256             # qkv/proj chunk width
NCH = T // CW        # 8 chunks per pair row-block

# sweep order: qh-major inside pairs so each pair's first q-half finishes
# early enough for transposes/projection to overlap the next sweeps
SWEEPS = [(0, 0), (1, 0), (0, 1), (1, 1),
          (2, 0), (3, 0), (2, 1), (3, 1),
          (4, 0), (5, 0), (4, 1), (5, 1)]

# kt indices whose exp runs on DVE (bit-trick) instead of ACT, per sweep
OFFLOAD = {si: (2, 5, 8, 11, 14) for si in range(1, 7)}
for si in (7, 8, 9):
    OFFLOAD[si] = (2, 5, 8, 11, 15)
OFFLOAD[10] = (2, 6, 10, 15)
OFFLOAD[11] = (2, 6, 10, 14)
OFFLOAD[0] = ()

# exp-as-fp16-bits: bits = trunc(A*s + B); bitcast int16 -> fp16
EXP_A = float(np.float32(1024.0 / np.log(2.0)))
EXP_B = float(np.float32(15 * 1024 - 58.7))

_prog_cache = {}
DEBUG_DUMP = False


def _build_program(repeat=1):
    import concourse.bass as bass
    import concourse.mybir as mybir
    import concourse.tile as tile
    from concourse import bacc

    f32 = mybir.dt.float32
    f32r = mybir.dt.float32r
    fp16 = mybir.dt.float16
    bf16 = mybir.dt.bfloat16
    i16 = mybir.dt.int16
    ACT_EXP = mybir.ActivationFunctionType.Exp
    ADD = mybir.AluOpType.add
    MULT = mybir.AluOpType.mult

    nc = bacc.Bacc()

    xt_d = nc.dram_tensor("xt", [EMBED, T], bf16, kind="ExternalInput")
    wq_d = nc.dram_tensor("wq", [EMBED, DL], bf16, kind="ExternalInput")
    wk_d = nc.dram_tensor("wk", [EMBED, DL], bf16, kind="ExternalInput")
    wv_d = nc.dram_tensor("wv", [EMBED, DL], bf16, kind="ExternalInput")
    bqs_d = nc.dram_tensor("bqs", [DL], f32, kind="ExternalInput")
    bk_d = nc.dram_tensor("bk", [DL], f32, kind="ExternalInput")
    ident_d = nc.dram_tensor("ident", [128, 128], fp16, kind="ExternalInput")
    wp_d = nc.dram_tensor("wp", [DL, EMBED], fp16, kind="ExternalInput")
    y_d = nc.dram_tensor("y", [T, EMBED], fp16, kind="ExternalOutput")

    with tile.TileContext(nc) as tc:
      for _rep in range(repeat):
        with tc.tile_pool(name="persist", bufs=1) as pers, \
             tc.tile_pool(name="qk", bufs=2) as qk_pool, \
             tc.tile_pool(name="pT", bufs=8) as pT_pool, \
             tc.tile_pool(name="pss", bufs=2, space="PSUM") as pss_pool, \
             tc.tile_pool(name="po", bufs=2, space="PSUM") as po_pool, \
             tc.tile_pool(name="pfix", bufs=1, space="PSUM") as pfix_pool:

            # ---- persistent SBUF ----
            xt_sb = pers.tile([128, NDT, T], bf16, name="xt_sb")
            wq_sb = pers.tile([128, NDT, DL], bf16, name="wq_sb")
            wk_sb = pers.tile([128, NDT, DL], bf16, name="wk_sb")
            wv_sb = pers.tile([128, NDT, DL], bf16, name="wv_sb")
            wp_sb = pers.tile([128, 3, EMBED], fp16, name="wp_sb")
            v_sb = pers.tile([128, NKT, HPC, HD], fp16, name="v_sb")
            O_sb = pers.tile([128, NQT, DL], fp16, name="O_sb")
            oT_sb = pers.tile([128, 3, T], fp16, name="oT_sb")
            y01_sb = pers.tile([128, NQT, EMBED], fp16, name="y01_sb")
            rcp_sb = pers.tile([128, 32], f32, name="rcp_sb")
            bqs_sb = pers.tile([128, 3], f32, name="bqs_sb")
            bk_sb = pers.tile([128, 3], f32, name="bk_sb")
            ones_sb = pers.tile([128, 1], fp16, name="ones_sb")
            ident_sb = pers.tile([128, 128], fp16, name="ident_sb")
            warm_sb = pers.tile([128, 512], f32r, name="warm_sb")

            # ---- persistent PSUM (1 bank each) ----
            den_ps = pfix_pool.tile([128, 32], f32, name="den_ps")
            fill_ps = pfix_pool.tile([128, 512], f32, name="fill_ps")

            nc.vector.memset(ones_sb, 1.0)
            nc.vector.memset(warm_sb.bitcast(f32), 0.0)

            # ---- input DMAs (transfer-serialized; order = priority) ----
            def xt_dma(c):
                nc.sync.dma_start(out=xt_sb[:, :, bass.ts(c, 512)],
                                  in_=xt_d.ap()[:, bass.ts(c, 512)].rearrange("(n p) m -> p n m", p=128))
            # HWDGE (sync) for everything: the DMA device serializes all
            # transfers anyway and HWDGE setup is far cheaper than SWDGE
            # descriptor generation for these many-descriptor patterns.
            # First quarter-chunk of xt + wq first so q-chunk 0 can start
            # as early as possible.
            nc.sync.dma_start(out=xt_sb[:, :, 0:256],
                              in_=xt_d.ap()[:, 0:256].rearrange("(n p) m -> p n m", p=128))
            nc.sync.dma_start(out=wq_sb, in_=wq_d.ap().rearrange("(n p) m -> p n m", p=128))
            nc.sync.dma_start(out=xt_sb[:, :, 256:512],
                              in_=xt_d.ap()[:, 256:512].rearrange("(n p) m -> p n m", p=128))
            nc.sync.dma_start(out=wk_sb, in_=wk_d.ap().rearrange("(n p) m -> p n m", p=128))
            nc.gpsimd.dma_start(out=bqs_sb, in_=bqs_d.ap().rearrange("(n p) -> p n", p=128))
            nc.gpsimd.dma_start(out=bk_sb, in_=bk_d.ap().rearrange("(n p) -> p n", p=128))
            nc.sync.dma_start(out=wv_sb, in_=wv_d.ap().rearrange("(n p) m -> p n m", p=128))
            xt_dma(1)
            xt_dma(2)
            xt_dma(3)
            nc.gpsimd.dma_start(out=ident_sb, in_=ident_d.ap())
            nc.sync.dma_start(out=wp_sb, in_=wp_d.ap().rearrange("(n p) m -> p n m", p=128))

            # ---- helpers ----
            qk_tiles = {}

            def warm(n):
                for _ in range(n):
                    psw = pss_pool.tile([128, 1024], f32, name="psw", tag="pss")
                    nc.tensor.matmul(psw[:, 0:512], warm_sb[0:2, 0:128],
                                     warm_sb[0:2, :], start=True, stop=True)

            def alloc_pair(p):
                qTp = qk_pool.tile([128, T], f32r, name="qTp", tag="qT")
                kTp = qk_pool.tile([128, T], f32r, name="kTp", tag="kT")
                qk_tiles[p] = (qTp, kTp)

            def qk_half(p, which, ch, half):
                # half-chunk of the q or k projection for pair p
                qTp, kTp = qk_tiles[p]
                w_sb = wq_sb if which == "q" else wk_sb
                reg = fill_ps[:, 0:CW] if which == "q" else fill_ps[:, CW:2 * CW]
                csl = bass.ds(ch * CW, CW)
                dts = range(0, 3) if half == 0 else range(3, NDT)
                for dt in dts:
                    nc.tensor.matmul(reg, w_sb[:, dt, bass.ts(p, 128)],
                                     xt_sb[:, dt, csl],
                                     start=(dt == 0), stop=(dt == NDT - 1))
                if half == 1:
                    if which == "q":
                        nc.vector.tensor_scalar(
                            out=qTp[:, csl], in0=reg,
                            scalar1=bqs_sb[:, p:p + 1], scalar2=float(SCALE),
                            op0=ADD, op1=MULT)
                    else:
                        nc.vector.tensor_scalar_add(
                            out=kTp[:, csl], in0=reg, scalar1=bk_sb[:, p:p + 1])

            def emit_v(kt):
                psv = fill_ps[:, 0:DL]
                for dt in range(NDT):
                    nc.tensor.matmul(psv, xt_sb[:, dt, bass.ts(kt, 128)],
                                     wv_sb[:, dt, :],
                                     start=(dt == 0), stop=(dt == NDT - 1))

            def vcopy(kt):
                nc.vector.tensor_copy(
                    out=v_sb[:, kt],
                    in_=fill_ps[:, 0:DL].rearrange("p (h d) -> p h d", h=HPC))

            pss_of = {}   # tick -> pss tile
            pT_of = {}    # tick -> pT tile
            Ot_of = {}    # sweep -> O psum tile

            def scores(i, tk):
                si, h, qh, kt = tk
                hp, off = h // 2, (h % 2) * 64
                qTp, kTp = qk_tiles[hp]
                pss = pss_pool.tile([128, 1024], f32, name="pss", tag="pss")
                pss_of[i] = pss
                for c2 in range(2):
                    nc.tensor.matmul(
                        pss[:, bass.ts(c2, 512)],
                        kTp[off:off + 64, bass.ts(kt, 128)],
                        qTp[off:off + 64, bass.ds(qh * 1024 + c2 * 512, 512)],
                        start=True, stop=True)

            def exp_tile(i, tk):
                si, h, qh, kt = tk
                pss = pss_of.pop(i)
                pT = pT_pool.tile([128, 1024], fp16, name="pT", tag="pT")
                pT_of[i] = pT
                if kt in OFFLOAD[si]:
                    nc.vector.tensor_scalar(
                        out=pT.bitcast(i16), in0=pss,
                        scalar1=EXP_A, scalar2=EXP_B, op0=MULT, op1=ADD)
                else:
                    nc.scalar.activation(out=pT, in_=pss, func=ACT_EXP)

            def av(i, tk):
                si, h, qh, kt = tk
                if si not in Ot_of:
                    Ot_of[si] = po_pool.tile([128, 8, HD], f32, name="O_t", tag="po")
                O_t = Ot_of[si]
                pT = pT_of.pop(i)
                dcol = (si % 4) * 8
                # one accumulation group per PSUM bank per sweep: start only
                # on the very first matmul touching the bank (pends the whole
                # 2KB zero region; first write to each sub-range zeroes it),
                # stop on the last
                for ql in range(8):
                    nc.tensor.matmul(O_t[:, ql, :], pT[:, bass.ts(ql, 128)],
                                     v_sb[:, kt, h, :],
                                     start=(kt == 0 and ql == 0),
                                     stop=(kt == NKT - 1 and ql == 7),
                                     skip_group_check=True)
                for ql in range(8):
                    nc.tensor.matmul(den_ps[:, dcol + ql:dcol + ql + 1],
                                     pT[:, bass.ts(ql, 128)], ones_sb[:, 0:1],
                                     start=(kt == 0 and ql == 0),
                                     stop=(kt == NKT - 1 and ql == 7),
                                     skip_group_check=True)

            # ---- deferred engine work queues ----
            dve_hookq = []   # normalize items, <=3 popped per tick
            dma_hookq = []   # transpose items, <=2 popped per tick

            # pair-half transpose triggers: sweep-idx -> (pair, qt range)
            TRANSP = {1: (0, range(0, 8)), 3: (0, range(8, 16)),
                      5: (1, range(0, 8)), 7: (1, range(8, 16)),
                      9: (2, range(0, 8)), 11: (2, range(8, 16))}

            def push_normalize(si):
                h, qh = SWEEPS[si]
                O_t = Ot_of.pop(si)
                dcol = (si % 4) * 8
                off = h * HD

                def rcp():
                    nc.vector.reciprocal(out=rcp_sb[:, dcol:dcol + 8],
                                         in_=den_ps[:, dcol:dcol + 8])
                dve_hookq.append(rcp)
                for ql in range(8):
                    qt = qh * 8 + ql

                    def mul(ql=ql, qt=qt):
                        nc.vector.tensor_scalar_mul(
                            out=O_sb[:, qt, off:off + HD], in0=O_t[:, ql, :],
                            scalar1=rcp_sb[:, dcol + ql:dcol + ql + 1])
                    dve_hookq.append(mul)
                if si in TRANSP:
                    pair, qts = TRANSP[si]

                    def push_tr(pair=pair, qts=qts):
                        for qt in qts:
                            def tr(pair=pair, qt=qt):
                                nc.sync.dma_start_transpose(
                                    out=oT_sb[:, pair, bass.ts(qt, 128)],
                                    in_=O_sb[:, qt, bass.ds(128 * pair, 128)])
                            dma_hookq.append(tr)
                    dve_hookq.append(push_tr)

            # ---- filler units ----
            ydma_done = set()

            psy_ctr = [0]

            def proj_unit(qt, nh, dts, copy="dve"):
                # psy ping-pong halves of fill_ps; copy: "dve" | "act" | None
                # (None = dead PE-filler unit: result abandoned in psum)
                r = psy_ctr[0] % 2
                psy_ctr[0] += 1
                reg = fill_ps[:, r * CW:r * CW + CW]
                for j, dtp in enumerate(dts):
                    nc.tensor.matmul(reg, oT_sb[:, dtp, bass.ts(qt, 128)],
                                     wp_sb[:, dtp, bass.ds(nh * CW, CW)],
                                     start=(j == 0), stop=(j == len(dts) - 1))
                if copy == "dve":
                    nc.vector.tensor_copy(
                        out=y01_sb[:, qt, bass.ds(nh * CW, CW)], in_=reg)
                elif copy == "act":
                    nc.scalar.copy(
                        out=y01_sb[:, qt, bass.ds(nh * CW, CW)], in_=reg)
                if copy is not None and nh == 2:
                    nc.sync.dma_start(
                        out=y_d.ap()[bass.ds(128 * qt, 128), :],
                        in_=y01_sb[:, qt, :])
                    ydma_done.add(qt)

            # build filler list: (avail, deadline, cost, fn, dead)
            # consumed strictly in order; deadline-forced when due.
            # NOTE: units that open psum accumulation groups in the shared
            # fill bank must not interleave with other fill-bank groups —
            # ordered consumption + adjacent halves guarantees that.
            def build_fillers():
                units = []
                # pair-0 chunks 4-7, JIT inside sweep 0 (full chunks so no
                # other fill-bank group can open between their halves)
                for ch in range(4, NCH):
                    units.append((0, 2 * ch - 2, 640,
                                  (lambda ch=ch: (qk_half(0, "k", ch, 0),
                                                  qk_half(0, "k", ch, 1))),
                                  False))
                for ch in range(4, NCH):
                    units.append((0, 28, 640,
                                  (lambda ch=ch: (qk_half(0, "q", ch, 0),
                                                  qk_half(0, "q", ch, 1))),
                                  False))
                for p, base in ((1, 64), (2, 128)):
                    avail = base - 48
                    for ch in range(NCH):
                        qdl = (base - 2) if ch < 4 else (base + 29)
                        kdl = (base - 2) if ch < 4 else (base + 2 * ch - 4)
                        for half in (0, 1):
                            units.append((avail, qdl, 320,
                                          (lambda p=p, ch=ch, half=half:
                                           qk_half(p, "q", ch, half)), False))
                        for half in (0, 1):
                            units.append((avail, kdl, 320,
                                          (lambda p=p, ch=ch, half=half:
                                           qk_half(p, "k", ch, half)), False))
                # dead PE-filler: dt0+dt1 partial projections, results
                # abandoned (the real 3-dt projection happens later);
                # qt0-7 oT available after sweep idx5's transposes,
                # qt8-15 after idx7's
                for qt in range(0, 8):
                    for nh in range(3):
                        units.append((104, 10 ** 9, 220,
                                      (lambda qt=qt, nh=nh:
                                       proj_unit(qt, nh, (0, 1), None)), True))
                for qt in range(8, 16):
                    for nh in range(3):
                        units.append((138, 10 ** 9, 220,
                                      (lambda qt=qt, nh=nh:
                                       proj_unit(qt, nh, (0, 1), None)), True))
                # full projection for qt 0-7 (pair-2 qh0 oT ready ~tick 166)
                for qt in range(0, 8):
                    for nh in range(3):
                        units.append((167, 10 ** 9, 700,
                                      (lambda qt=qt, nh=nh:
                                       proj_unit(qt, nh, (0, 1, 2), "dve")),
                                      False))
                return units

            # ================= program =================
            alloc_pair(0)
            warm(10)
            for ch in (0, 1):
                qk_half(0, "q", ch, 0)
                qk_half(0, "q", ch, 1)
            warm(1)
            for ch in (0, 1):
                qk_half(0, "k", ch, 0)
                qk_half(0, "k", ch, 1)
            # V for kpos tiles 0-3 here: real PE work (needs only xt chunk 0
            # and wv) in the window where q2/q3-k2/k3 would stall on the
            # xt chunk-1 DMA
            for kt in range(4):
                emit_v(kt)
                vcopy(kt)
                if kt < 3:
                    warm(1)
            for ch in (2, 3):
                qk_half(0, "q", ch, 0)
                qk_half(0, "q", ch, 1)
            for ch in (2, 3):
                qk_half(0, "k", ch, 0)
                qk_half(0, "k", ch, 1)

            ticks = [(si, h, qh, kt)
                     for si, (h, qh) in enumerate(SWEEPS) for kt in range(NKT)]
            units = build_fillers()
            alloc_ticks = {1: 16, 2: 80}
            uidx = 0
            spent = 0.0
            total_cost = sum(u[2] for u in units)
            BUDGET = total_cost / len(ticks)

            # prologue of the pipeline
            scores(0, ticks[0])
            exp_tile(0, ticks[0])

            if DEBUG_DUMP:
                dbg_p0_sb = pers.tile([128, 1024], fp16, name="dbg_p0_sb")
                nc.vector.tensor_copy(out=dbg_p0_sb, in_=pT_of[0])

            for i, tk in enumerate(ticks):
                si, h, qh, kt = tk
                for p, at in alloc_ticks.items():
                    if at == i and p not in qk_tiles:
                        alloc_pair(p)
                if i >= 2:
                    av(i - 2, ticks[i - 2])
                    psi = ticks[i - 2][0]
                    if ticks[i - 2][3] == NKT - 1:
                        push_normalize(psi)
                # deferred DVE / DMA hook items
                for _ in range(2):
                    if dve_hookq:
                        dve_hookq.pop(0)()
                for _ in range(3):
                    if dma_hookq:
                        dma_hookq.pop(0)()
                # paced fillers (ordered; deadline-forced when due)
                while uidx < len(units) and units[uidx][1] <= i:
                    spent += units[uidx][2]
                    units[uidx][3]()
                    uidx += 1
                while uidx < len(units) and units[uidx][0] <= i:
                    if units[uidx][4] and i >= 166:
                        uidx += 1   # drop stale dead-filler
                        continue
                    if spent >= (i + 1) * BUDGET:
                        break
                    spent += units[uidx][2]
                    units[uidx][3]()
                    uidx += 1
                # V emission four ticks ahead (kt 0-3 were done in the
                # prologue) so its DVE copy never gates a fill-bank group
                if si == 0 and kt + 4 < NKT:
                    emit_v(kt + 4)
                    vcopy(kt + 4)
                # scores + exp emitted together, one tick ahead: maximizes
                # the lead the ACT/DVE exp has over its AV consumer
                if i + 1 < len(ticks):
                    scores(i + 1, ticks[i + 1])
                    exp_tile(i + 1, ticks[i + 1])

            # ---- tail ----
            n = len(ticks)
            av(n - 2, ticks[n - 2])
            av(n - 1, ticks[n - 1])
            # leftover (non-dead) fillers first: they use the fill bank,
            # which the tail transposes then reuse
            while uidx < len(units):
                if not units[uidx][4]:
                    units[uidx][3]()
                uidx += 1
            while dve_hookq:
                dve_hookq.pop(0)()
            while dma_hookq:
                dma_hookq.pop(0)()
            # last sweep (5,1): normalize + PE-transpose + project, chained
            # per q-tile to minimize the drain. Transposes go through the PE
            # (identity matmul) instead of DMA: ~53ns each and no 3us DMA
            # round-trip on the critical path.
            O_t11 = Ot_of.pop(11)
            dcol = (11 % 4) * 8
            nc.vector.reciprocal(out=rcp_sb[:, dcol:dcol + 8],
                                 in_=den_ps[:, dcol:dcol + 8])
            trp_t = po_pool.tile([128, 8, HD], f32, name="trp", tag="po")
            tail_ps = [pss_pool.tile([128, 1024], f32, name="tailps", tag="pss")
                       for _ in range(2)]
            j = 0
            for ql in range(8):
                qt = 8 + ql
                nc.vector.tensor_scalar_mul(
                    out=O_sb[:, qt, 320:384], in0=O_t11[:, ql, :],
                    scalar1=rcp_sb[:, dcol + ql:dcol + ql + 1])
                tr_out = trp_t[:, ql, :].bitcast(fp16)
                nc.tensor.transpose(tr_out, O_sb[:, qt, 256:384], ident_sb)
                if ql % 2:
                    nc.vector.tensor_copy(out=oT_sb[:, 2, bass.ts(qt, 128)],
                                          in_=tr_out)
                else:
                    nc.scalar.copy(out=oT_sb[:, 2, bass.ts(qt, 128)],
                                   in_=tr_out)
                for nh in range(3):
                    reg = tail_ps[(j // 4) % 2][:, bass.ds((j % 4) * CW, CW)]
                    j += 1
                    for dtp in (0, 1, 2):
                        nc.tensor.matmul(reg, oT_sb[:, dtp, bass.ts(qt, 128)],
                                         wp_sb[:, dtp, bass.ds(nh * CW, CW)],
                                         start=(dtp == 0), stop=(dtp == 2),
                                         skip_group_check=True)
                    dst = y01_sb[:, qt, bass.ds(nh * CW, CW)]
                    if j % 2:
                        nc.vector.tensor_copy(out=dst, in_=reg)
                    else:
                        nc.scalar.copy(out=dst, in_=reg)
                nc.sync.dma_start(out=y_d.ap()[bass.ds(128 * qt, 128), :],
                                  in_=y01_sb[:, qt, :])
                ydma_done.add(qt)
            for qt in range(16):
                if qt not in ydma_done:
                    nc.sync.dma_start(out=y_d.ap()[bass.ds(128 * qt, 128), :],
                                      in_=y01_sb[:, qt, :])
                    ydma_done.add(qt)

            if DEBUG_DUMP:
                den_d = nc.dram_tensor("d_den", [128, 32], f32, kind="ExternalOutput")
                den_cp = pers.tile([128, 32], f32, name="den_cp")
                nc.vector.tensor_copy(out=den_cp, in_=den_ps)
                nc.sync.dma_start(out=den_d.ap(), in_=den_cp)
                qT0, kT0 = qk_tiles[0]
                dumps = [("d_qT0", qT0, f32), ("d_kT0", kT0, f32),
                         ("d_v", v_sb, fp16), ("d_O", O_sb, fp16),
                         ("d_oT", oT_sb, fp16), ("d_rcp", rcp_sb, f32),
                         ("d_p0", dbg_p0_sb, fp16)]
                for nm, t, dt_ in dumps:
                    sh = [t.shape[0], int(np.prod(t.shape[1:]))]
                    dd = nc.dram_tensor(nm, sh, t.dtype, kind="ExternalOutput")
                    nc.sync.dma_start(out=dd.ap(), in_=t.rearrange(
                        "p a b -> p (a b)") if len(t.shape) == 3 else (
                        t.rearrange("p a b c -> p (a b c)") if len(t.shape) == 4 else t))

    nc.finalize()
    return nc


def _shard_inputs(x, w_qkv, b_qkv, w_proj):
    import ml_dtypes
    bf16 = ml_dtypes.bfloat16
    in_maps = []
    for c in range(NCORES):
        b, g = c // 2, c % 2
        sl = slice(DL * g, DL * g + DL)
        in_maps.append({
            "xt": np.ascontiguousarray(x[b].T).astype(bf16),
            "wq": np.ascontiguousarray(w_qkv[:, sl]).astype(bf16),
            "wk": np.ascontiguousarray(w_qkv[:, EMBED:][:, sl]).astype(bf16),
            "wv": np.ascontiguousarray(w_qkv[:, 2 * EMBED:][:, sl]).astype(bf16),
            "bqs": np.ascontiguousarray(b_qkv[sl]).astype(np.float32),
            "bk": np.ascontiguousarray(b_qkv[EMBED:][sl]).astype(np.float32),
            "wp": np.ascontiguousarray(w_proj[sl, :]).astype(np.float16),
            "ident": np.eye(128, dtype=np.float16),
        })
    return in_maps


def kernel(x, w_qkv, b_qkv, w_proj, b_proj, _profile=False, _repeat=1):
    from concourse.bass_utils import run_bass_kernel_spmd

    x = np.asarray(x, dtype=np.float32)
    w_qkv = np.asarray(w_qkv, dtype=np.float32)
    b_qkv = np.asarray(b_qkv, dtype=np.float32)
    w_proj = np.asarray(w_proj, dtype=np.float32)
    b_proj = np.asarray(b_proj, dtype=np.float32)

    if _repeat not in _prog_cache:
        _prog_cache[_repeat] = _build_program(_repeat)
    nc = _prog_cache[_repeat]

    in_maps = _shard_inputs(x, w_qkv, b_qkv, w_proj)
    res = run_bass_kernel_spmd(
        nc, in_maps, list(range(NCORES)), trace=_profile,
    )

    # host-side gather: sum the two head-group partials per batch and add
    # the bias row (v-bias folded through w_proj, plus b_proj itself)
    bias_row = b_qkv[2 * EMBED:] @ w_proj + b_proj
    y = np.empty((B, T, EMBED), dtype=np.float32)
    for b in range(B):
        y[b] = (np.asarray(res.results[2 * b]["y"], dtype=np.float32)
                + np.asarray(res.results[2 * b + 1]["y"], dtype=np.float32)
                + bias_row)
    if _profile:
        return y, res
    return y


# revision 57
# speedup vs baseline: 1.0037x; 1.0037x over previous
"""Multi-head attention (B=4, T=2048, D=768, H=12) on 8 NeuronCores.

Sharding: core c handles batch b = c//2 and head-group g = c%2 (heads
6g..6g+5).  Each core computes its 6 heads' attention and a partial
output projection; the host sums the two partials per batch and adds
the bias terms (v-bias folds through w_proj since softmax rows sum to 1).

Device formulation (cost-model-shaped: every matmul streams with full
128-wide output partitions; PE is kept continuously busy so the p-state
stays at 2.4 GHz):

  qT/kT [384, 2048] = W.T @ xT  (bf16 inputs, f32 psum, f32r qT/kT)
  S^T [kpos 128, q] = kT_h.T @ qT_h    per (head, kpos-tile)
  P^T = exp(S^T)  -- ACT engine, or offloaded to DVE via the
        exp-as-int-bitcast trick (P in fp16)
  O [q 128, 64] += P^T_chunk.T @ v_h   accumulated over kpos tiles
  den[q, 1]    += P^T_chunk.T @ ones
  O_sb = O * (1/den)  (DVE per-partition scalar), fp16
  oT = DMA-crossbar transpose of O_sb per (pair, q-tile)
  y = oT.T @ wp  partial, fp16, host adds partner core + bias row

Schedule: a flat software pipeline over 12 sweeps x 16 kpos-tiles
(ticks): scores(t+1) | exp(t) | AV(t-2), with QKV projections of later
head-pairs, the output projection, and V emission injected as PE filler
paced by deadlines.  A fraction of exp tiles per sweep runs on DVE so
ACT never gates PE.
"""

import numpy as np

EMBED = 768
HEADS = 12
HD = 64
SCALE = HD ** -0.5
B, T = 4, 2048
NCORES = 8
HPC = 6            # heads per core
DL = HPC * HD      # 384 local model dims per core

NDT = EMBED // 128   # 6 contraction tiles over embed dim
NKT = T // 128       # 16 key-position tiles
NQT = T // 128       # 16 query row tiles
CW = 256             # qkv/proj chunk width
NCH = T // CW        # 8 chunks per pair row-block

# sweep order: qh-major inside pairs so each pair's first q-half finishes
# early enough for transposes/projection to overlap the next sweeps
SWEEPS = [(0, 0), (1, 0), (0, 1), (1, 1),
          (2, 0), (3, 0), (2, 1), (3, 1),
          (4, 0), (5, 0), (4, 1), (5, 1)]

# kt indices whose exp runs on DVE (bit-trick) instead of ACT, per sweep
OFFLOAD = {si: (2, 5, 8, 11, 14) for si in range(1, 7)}
for si in (7, 8, 9):
    OFFLOAD[si] = (2, 5, 8, 11, 15)
OFFLOAD[10] = (2, 6, 10, 15)
OFFLOAD[11] = (2, 6, 10, 14)
OFFLOAD[0] = ()

# exp-as-fp16-bits: bits = trunc(A*s + B); bitcast int16 -> fp16
EXP_A = float(np.float32(1024.0 / np.log(2.0)))
EXP_B = float(np.float32(15 * 1024 - 58.7))

_prog_cache = {}
DEBUG_DUMP = False


def _build_program(repeat=1):
    import concourse.bass as bass
    import concourse.mybir as mybir
    import concourse.tile as tile
    from concourse import bacc

    f32 = mybir.dt.float32
    f32r = mybir.dt.float32r
    fp16 = mybir.dt.float16
    bf16 = mybir.dt.bfloat16
    i16 = mybir.dt.int16
    ACT_EXP = mybir.ActivationFunctionType.Exp
    ADD = mybir.AluOpType.add
    MULT = mybir.AluOpType.mult

    nc = bacc.Bacc()

    xt_d = nc.dram_tensor("xt", [EMBED, T], bf16, kind="ExternalInput")
    wq_d = nc.dram_tensor("wq", [EMBED, DL], bf16, kind="ExternalInput")
    wk_d = nc.dram_tensor("wk", [EMBED, DL], bf16, kind="ExternalInput")
    wv_d = nc.dram_tensor("wv", [EMBED, DL], bf16, kind="ExternalInput")
    bqs_d = nc.dram_tensor("bqs", [DL], f32, kind="ExternalInput")
    bk_d = nc.dram_tensor("bk", [DL], f32, kind="ExternalInput")
    ident_d = nc.dram_tensor("ident", [128, 128], fp16, kind="ExternalInput")
    wp_d = nc.dram_tensor("wp", [DL, EMBED], fp16, kind="ExternalInput")
    y_d = nc.dram_tensor("y", [T, EMBED], fp16, kind="ExternalOutput")

    with tile.TileContext(nc) as tc:
      for _rep in range(repeat):
        with tc.tile_pool(name="persist", bufs=1) as pers, \
             tc.tile_pool(name="qk", bufs=2) as qk_pool, \
             tc.tile_pool(name="pT", bufs=8) as pT_pool, \
             tc.tile_pool(name="pss", bufs=2, space="PSUM") as pss_pool, \
             tc.tile_pool(name="po", bufs=2, space="PSUM") as po_pool, \
             tc.tile_pool(name="pfix", bufs=1, space="PSUM") as pfix_pool:

            # ---- persistent SBUF ----
            xt_sb = pers.tile([128, NDT, T], bf16, name="xt_sb")
            wq_sb = pers.tile([128, NDT, DL], bf16, name="wq_sb")
            wk_sb = pers.tile([128, NDT, DL], bf16, name="wk_sb")
            wv_sb = pers.tile([128, NDT, DL], bf16, name="wv_sb")
            wp_sb = pers.tile([128, 3, EMBED], fp16, name="wp_sb")
            v_sb = pers.tile([128, NKT, HPC, HD], fp16, name="v_sb")
            O_sb = pers.tile([128, NQT, DL], fp16, name="O_sb")
            oT_sb = pers.tile([128, 3, T], fp16, name="oT_sb")
            y01_sb = pers.tile([128, NQT, EMBED], fp16, name="y01_sb")
            rcp_sb = pers.tile([128, 32], f32, name="rcp_sb")
            bqs_sb = pers.tile([128, 3], f32, name="bqs_sb")
            bk_sb = pers.tile([128, 3], f32, name="bk_sb")
            ones_sb = pers.tile([128, 1], fp16, name="ones_sb")
            ident_sb = pers.tile([128, 128], fp16, name="ident_sb")
            warm_sb = pers.tile([128, 512], f32r, name="warm_sb")

            # ---- persistent PSUM (1 bank each) ----
            den_ps = pfix_pool.tile([128, 32], f32, name="den_ps")
            fill_ps = pfix_pool.tile([128, 512], f32, name="fill_ps")

            nc.vector.memset(ones_sb, 1.0)
            nc.vector.memset(warm_sb.bitcast(f32), 0.0)

            # ---- input DMAs (transfer-serialized; order = priority) ----
            def xt_dma(c):
                nc.sync.dma_start(out=xt_sb[:, :, bass.ts(c, 512)],
                                  in_=xt_d.ap()[:, bass.ts(c, 512)].rearrange("(n p) m -> p n m", p=128))
            # HWDGE (sync) for everything: the DMA device serializes all
            # transfers anyway and HWDGE setup is far cheaper than SWDGE
            # descriptor generation for these many-descriptor patterns.
            # First quarter-chunk of xt + wq first so q-chunk 0 can start
            # as early as possible.
            nc.sync.dma_start(out=xt_sb[:, :, 0:256],
                              in_=xt_d.ap()[:, 0:256].rearrange("(n p) m -> p n m", p=128))
            nc.sync.dma_start(out=wq_sb, in_=wq_d.ap().rearrange("(n p) m -> p n m", p=128))
            nc.sync.dma_start(out=xt_sb[:, :, 256:512],
                              in_=xt_d.ap()[:, 256:512].rearrange("(n p) m -> p n m", p=128))
            nc.sync.dma_start(out=wk_sb, in_=wk_d.ap().rearrange("(n p) m -> p n m", p=128))
            nc.gpsimd.dma_start(out=bqs_sb, in_=bqs_d.ap().rearrange("(n p) -> p n", p=128))
            nc.gpsimd.dma_start(out=bk_sb, in_=bk_d.ap().rearrange("(n p) -> p n", p=128))
            nc.sync.dma_start(out=wv_sb, in_=wv_d.ap().rearrange("(n p) m -> p n m", p=128))
            xt_dma(1)
            xt_dma(2)
            xt_dma(3)
            nc.gpsimd.dma_start(out=ident_sb, in_=ident_d.ap())
            nc.sync.dma_start(out=wp_sb, in_=wp_d.ap().rearrange("(n p) m -> p n m", p=128))

            # ---- helpers ----
            qk_tiles = {}

            def warm(n):
                for _ in range(n):
                    psw = pss_pool.tile([128, 1024], f32, name="psw", tag="pss")
                    nc.tensor.matmul(psw[:, 0:512], warm_sb[0:2, 0:128],
                                     warm_sb[0:2, :], start=True, stop=True)

            def alloc_pair(p):
                qTp = qk_pool.tile([128, T], f32r, name="qTp", tag="qT")
                kTp = qk_pool.tile([128, T], f32r, name="kTp", tag="kT")
                qk_tiles[p] = (qTp, kTp)

            def qk_half(p, which, ch, half):
                # half-chunk of the q or k projection for pair p
                qTp, kTp = qk_tiles[p]
                w_sb = wq_sb if which == "q" else wk_sb
                reg = fill_ps[:, 0:CW] if which == "q" else fill_ps[:, CW:2 * CW]
                csl = bass.ds(ch * CW, CW)
                dts = range(0, 3) if half == 0 else range(3, NDT)
                for dt in dts:
                    nc.tensor.matmul(reg, w_sb[:, dt, bass.ts(p, 128)],
                                     xt_sb[:, dt, csl],
                                     start=(dt == 0), stop=(dt == NDT - 1))
                if half == 1:
                    if which == "q":
                        nc.vector.tensor_scalar(
                            out=qTp[:, csl], in0=reg,
                            scalar1=bqs_sb[:, p:p + 1], scalar2=float(SCALE),
                            op0=ADD, op1=MULT)
                    else:
                        nc.vector.tensor_scalar_add(
                            out=kTp[:, csl], in0=reg, scalar1=bk_sb[:, p:p + 1])

            def emit_v(kt):
                psv = fill_ps[:, 0:DL]
                for dt in range(NDT):
                    nc.tensor.matmul(psv, xt_sb[:, dt, bass.ts(kt, 128)],
                                     wv_sb[:, dt, :],
                                     start=(dt == 0), stop=(dt == NDT - 1))

            def vcopy(kt):
                nc.vector.tensor_copy(
                    out=v_sb[:, kt],
                    in_=fill_ps[:, 0:DL].rearrange("p (h d) -> p h d", h=HPC))

            pss_of = {}   # tick -> pss tile
            pT_of = {}    # tick -> pT tile
            Ot_of = {}    # sweep -> O psum tile

            def scores(i, tk):
                si, h, qh, kt = tk
                hp, off = h // 2, (h % 2) * 64
                qTp, kTp = qk_tiles[hp]
                pss = pss_pool.tile([128, 1024], f32, name="pss", tag="pss")
                pss_of[i] = pss
                for c2 in range(2):
                    nc.tensor.matmul(
                        pss[:, bass.ts(c2, 512)],
                        kTp[off:off + 64, bass.ts(kt, 128)],
                        qTp[off:off + 64, bass.ds(qh * 1024 + c2 * 512, 512)],
                        start=True, stop=True)

            def exp_tile(i, tk):
                si, h, qh, kt = tk
                pss = pss_of.pop(i)
                pT = pT_pool.tile([128, 1024], fp16, name="pT", tag="pT")
                pT_of[i] = pT
                if kt in OFFLOAD[si]:
                    nc.vector.tensor_scalar(
                        out=pT.bitcast(i16), in0=pss,
                        scalar1=EXP_A, scalar2=EXP_B, op0=MULT, op1=ADD)
                else:
                    nc.scalar.activation(out=pT, in_=pss, func=ACT_EXP)

            def av(i, tk):
                si, h, qh, kt = tk
                if si not in Ot_of:
                    Ot_of[si] = po_pool.tile([128, 8, HD], f32, name="O_t", tag="po")
                O_t = Ot_of[si]
                pT = pT_of.pop(i)
                dcol = (si % 4) * 8
                # one accumulation group per PSUM bank per sweep: start only
                # on the very first matmul touching the bank (pends the whole
                # 2KB zero region; first write to each sub-range zeroes it),
                # stop on the last
                for ql in range(8):
                    nc.tensor.matmul(O_t[:, ql, :], pT[:, bass.ts(ql, 128)],
                                     v_sb[:, kt, h, :],
                                     start=(kt == 0 and ql == 0),
                                     stop=(kt == NKT - 1 and ql == 7),
                                     skip_group_check=True)
                for ql in range(8):
                    nc.tensor.matmul(den_ps[:, dcol + ql:dcol + ql + 1],
                                     pT[:, bass.ts(ql, 128)], ones_sb[:, 0:1],
                                     start=(kt == 0 and ql == 0),
                                     stop=(kt == NKT - 1 and ql == 7),
                                     skip_group_check=True)

            # ---- deferred engine work queues ----
            dve_hookq = []   # normalize items, <=3 popped per tick
            dma_hookq = []   # transpose items, <=2 popped per tick

            # pair-half transpose triggers: sweep-idx -> (pair, qt range)
            TRANSP = {1: (0, range(0, 8)), 3: (0, range(8, 16)),
                      5: (1, range(0, 8)), 7: (1, range(8, 16)),
                      9: (2, range(0, 8)), 11: (2, range(8, 16))}

            def push_normalize(si):
                h, qh = SWEEPS[si]
                O_t = Ot_of.pop(si)
                dcol = (si % 4) * 8
                off = h * HD

                def rcp():
                    nc.vector.reciprocal(out=rcp_sb[:, dcol:dcol + 8],
                                         in_=den_ps[:, dcol:dcol + 8])
                dve_hookq.append(rcp)
                for ql in range(8):
                    qt = qh * 8 + ql

                    def mul(ql=ql, qt=qt):
                        nc.vector.tensor_scalar_mul(
                            out=O_sb[:, qt, off:off + HD], in0=O_t[:, ql, :],
                            scalar1=rcp_sb[:, dcol + ql:dcol + ql + 1])
                    dve_hookq.append(mul)
                if si in TRANSP:
                    pair, qts = TRANSP[si]

                    def push_tr(pair=pair, qts=qts):
                        for qt in qts:
                            def tr(pair=pair, qt=qt):
                                nc.sync.dma_start_transpose(
                                    out=oT_sb[:, pair, bass.ts(qt, 128)],
                                    in_=O_sb[:, qt, bass.ds(128 * pair, 128)])
                            dma_hookq.append(tr)
                    dve_hookq.append(push_tr)

            # ---- filler units ----
            ydma_done = set()

            psy_ctr = [0]

            def proj_unit(qt, nh, dts, copy="dve"):
                # psy ping-pong halves of fill_ps; copy: "dve" | "act" | None
                # (None = dead PE-filler unit: result abandoned in psum)
                r = psy_ctr[0] % 2
                psy_ctr[0] += 1
                reg = fill_ps[:, r * CW:r * CW + CW]
                for j, dtp in enumerate(dts):
                    nc.tensor.matmul(reg, oT_sb[:, dtp, bass.ts(qt, 128)],
                                     wp_sb[:, dtp, bass.ds(nh * CW, CW)],
                                     start=(j == 0), stop=(j == len(dts) - 1))
                if copy == "dve":
                    nc.vector.tensor_copy(
                        out=y01_sb[:, qt, bass.ds(nh * CW, CW)], in_=reg)
                elif copy == "act":
                    nc.scalar.copy(
                        out=y01_sb[:, qt, bass.ds(nh * CW, CW)], in_=reg)
                if copy is not None and nh == 2:
                    nc.sync.dma_start(
                        out=y_d.ap()[bass.ds(128 * qt, 128), :],
                        in_=y01_sb[:, qt, :])
                    ydma_done.add(qt)

            # build filler list: (avail, deadline, cost, fn, dead)
            # consumed strictly in order; deadline-forced when due.
            # NOTE: units that open psum accumulation groups in the shared
            # fill bank must not interleave with other fill-bank groups —
            # ordered consumption + adjacent halves guarantees that.
            def build_fillers():
                units = []
                # pair-0 chunks 4-7, JIT inside sweep 0 (full chunks so no
                # other fill-bank group can open between their halves)
                for ch in range(4, NCH):
                    units.append((0, 2 * ch - 2, 640,
                                  (lambda ch=ch: (qk_half(0, "k", ch, 0),
                                                  qk_half(0, "k", ch, 1))),
                                  False))
                for ch in range(4, NCH):
                    units.append((0, 28, 640,
                                  (lambda ch=ch: (qk_half(0, "q", ch, 0),
                                                  qk_half(0, "q", ch, 1))),
                                  False))
                for p, base in ((1, 64), (2, 128)):
                    avail = base - 48
                    for ch in range(NCH):
                        qdl = (base - 2) if ch < 4 else (base + 29)
                        kdl = (base - 2) if ch < 4 else (base + 2 * ch - 4)
                        for half in (0, 1):
                            units.append((avail, qdl, 320,
                                          (lambda p=p, ch=ch, half=half:
                                           qk_half(p, "q", ch, half)), False))
                        for half in (0, 1):
                            units.append((avail, kdl, 320,
                                          (lambda p=p, ch=ch, half=half:
                                           qk_half(p, "k", ch, half)), False))
                # dead PE-filler: dt0+dt1 partial projections, results
                # abandoned (the real 3-dt projection happens later);
                # qt0-7 oT available after sweep idx5's transposes,
                # qt8-15 after idx7's
                for qt in range(0, 8):
                    for nh in range(3):
                        units.append((104, 10 ** 9, 220,
                                      (lambda qt=qt, nh=nh:
                                       proj_unit(qt, nh, (0, 1), None)), True))
                for qt in range(8, 16):
                    for nh in range(3):
                        units.append((138, 10 ** 9, 220,
                                      (lambda qt=qt, nh=nh:
                                       proj_unit(qt, nh, (0, 1), None)), True))
                # full projection for qt 0-7 (pair-2 qh0 oT ready ~tick 166)
                for qt in range(0, 8):
                    for nh in range(3):
                        units.append((167, 10 ** 9, 700,
                                      (lambda qt=qt, nh=nh:
                                       proj_unit(qt, nh, (0, 1, 2), "dve")),
                                      False))
                return units

            # ================= program =================
            alloc_pair(0)
            warm(10)
            for ch in (0, 1):
                qk_half(0, "q", ch, 0)
                qk_half(0, "q", ch, 1)
            warm(1)
            for ch in (0, 1):
                qk_half(0, "k", ch, 0)
                qk_half(0, "k", ch, 1)
            # V for kpos tiles 0-3 here: real PE work in the window where
            # the later xt/weight DMAs would otherwise leave PE starved
            emit_v(0)
            vcopy(0)
            emit_v(1)
            vcopy(1)
            for ch in (2, 3):
                qk_half(0, "q", ch, 0)
                qk_half(0, "q", ch, 1)
            emit_v(2)
            vcopy(2)
            for ch in (2, 3):
                qk_half(0, "k", ch, 0)
                qk_half(0, "k", ch, 1)
            emit_v(3)
            vcopy(3)

            ticks = [(si, h, qh, kt)
                     for si, (h, qh) in enumerate(SWEEPS) for kt in range(NKT)]
            units = build_fillers()
            alloc_ticks = {1: 16, 2: 80}
            uidx = 0
            spent = 0.0
            total_cost = sum(u[2] for u in units)
            BUDGET = total_cost / len(ticks)

            # prologue of the pipeline
            scores(0, ticks[0])
            exp_tile(0, ticks[0])

            if DEBUG_DUMP:
                dbg_p0_sb = pers.tile([128, 1024], fp16, name="dbg_p0_sb")
                nc.vector.tensor_copy(out=dbg_p0_sb, in_=pT_of[0])

            for i, tk in enumerate(ticks):
                si, h, qh, kt = tk
                for p, at in alloc_ticks.items():
                    if at == i and p not in qk_tiles:
                        alloc_pair(p)
                if i >= 2:
                    av(i - 2, ticks[i - 2])
                    psi = ticks[i - 2][0]
                    if ticks[i - 2][3] == NKT - 1:
                        push_normalize(psi)
                # deferred DVE / DMA hook items
                for _ in range(2):
                    if dve_hookq:
                        dve_hookq.pop(0)()
                for _ in range(3):
                    if dma_hookq:
                        dma_hookq.pop(0)()
                # paced fillers (ordered; deadline-forced when due)
                while uidx < len(units) and units[uidx][1] <= i:
                    spent += units[uidx][2]
                    units[uidx][3]()
                    uidx += 1
                while uidx < len(units) and units[uidx][0] <= i:
                    if units[uidx][4] and i >= 166:
                        uidx += 1   # drop stale dead-filler
                        continue
                    if spent >= (i + 1) * BUDGET:
                        break
                    spent += units[uidx][2]
                    units[uidx][3]()
                    uidx += 1
                # V emission four ticks ahead (kt 0-3 were done in the
                # prologue) so its DVE copy never gates a fill-bank group
                if si == 0 and kt + 4 < NKT:
                    emit_v(kt + 4)
                    vcopy(kt + 4)
                # scores + exp emitted together, one tick ahead: maximizes
                # the lead the ACT/DVE exp has over its AV consumer
                if i + 1 < len(ticks):
                    scores(i + 1, ticks[i + 1])
                    exp_tile(i + 1, ticks[i + 1])

            # ---- tail ----
            n = len(ticks)
            av(n - 2, ticks[n - 2])
            av(n - 1, ticks[n - 1])
            # leftover (non-dead) fillers first: they use the fill bank,
            # which the tail transposes then reuse
            while uidx < len(units):
                if not units[uidx][4]:
                    units[uidx][3]()
                uidx += 1
            while dve_hookq:
                dve_hookq.pop(0)()
            while dma_hookq:
                dma_hookq.pop(0)()
            # last sweep (5,1): normalize + PE-transpose + project, chained
            # per q-tile to minimize the drain. Transposes go through the PE
            # (identity matmul) instead of DMA: ~53ns each and no 3us DMA
            # round-trip on the critical path.
            O_t11 = Ot_of.pop(11)
            dcol = (11 % 4) * 8
            nc.vector.reciprocal(out=rcp_sb[:, dcol:dcol + 8],
                                 in_=den_ps[:, dcol:dcol + 8])
            trp_t = po_pool.tile([128, 8, HD], f32, name="trp", tag="po")
            tail_ps = [pss_pool.tile([128, 1024], f32, name="tailps", tag="pss")
                       for _ in range(2)]
            j = 0
            for ql in range(8):
                qt = 8 + ql
                nc.vector.tensor_scalar_mul(
                    out=O_sb[:, qt, 320:384], in0=O_t11[:, ql, :],
                    scalar1=rcp_sb[:, dcol + ql:dcol + ql + 1])
                tr_out = trp_t[:, ql, :].bitcast(fp16)
                nc.tensor.transpose(tr_out, O_sb[:, qt, 256:384], ident_sb)
                if ql % 2:
                    nc.vector.tensor_copy(out=oT_sb[:, 2, bass.ts(qt, 128)],
                                          in_=tr_out)
                else:
                    nc.scalar.copy(out=oT_sb[:, 2, bass.ts(qt, 128)],
                                   in_=tr_out)
                for nh in range(3):
                    reg = tail_ps[(j // 4) % 2][:, bass.ds((j % 4) * CW, CW)]
                    j += 1
                    for dtp in (0, 1, 2):
                        nc.tensor.matmul(reg, oT_sb[:, dtp, bass.ts(qt, 128)],
                                         wp_sb[:, dtp, bass.ds(nh * CW, CW)],
                                         start=(dtp == 0), stop=(dtp == 2),
                                         skip_group_check=True)
                    dst = y01_sb[:, qt, bass.ds(nh * CW, CW)]
                    if j % 2:
                        nc.vector.tensor_copy(out=dst, in_=reg)
                    else:
                        nc.scalar.copy(out=dst, in_=reg)
                nc.sync.dma_start(out=y_d.ap()[bass.ds(128 * qt, 128), :],
                                  in_=y01_sb[:, qt, :])
                ydma_done.add(qt)
            for qt in range(16):
                if qt not in ydma_done:
                    nc.sync.dma_start(out=y_d.ap()[bass.ds(128 * qt, 128), :],
                                      in_=y01_sb[:, qt, :])
                    ydma_done.add(qt)

            if DEBUG_DUMP:
                den_d = nc.dram_tensor("d_den", [128, 32], f32, kind="ExternalOutput")
                den_cp = pers.tile([128, 32], f32, name="den_cp")
                nc.vector.tensor_copy(out=den_cp, in_=den_ps)
                nc.sync.dma_start(out=den_d.ap(), in_=den_cp)
                qT0, kT0 = qk_tiles[0]
                dumps = [("d_qT0", qT0, f32), ("d_kT0", kT0, f32),
                         ("d_v", v_sb, fp16), ("d_O", O_sb, fp16),
                         ("d_oT", oT_sb, fp16), ("d_rcp", rcp_sb, f32),
                         ("d_p0", dbg_p0_sb, fp16)]
                for nm, t, dt_ in dumps:
                    sh = [t.shape[0], int(np.prod(t.shape[1:]))]
                    dd = nc.dram_tensor(nm, sh, t.dtype, kind="ExternalOutput")
                    nc.sync.dma_start(out=dd.ap(), in_=t.rearrange(
                        "p a b -> p (a b)") if len(t.shape) == 3 else (
                        t.rearrange("p a b c -> p (a b c)") if len(t.shape) == 4 else t))

    nc.finalize()
    return nc


def _shard_inputs(x, w_qkv, b_qkv, w_proj):
    import ml_dtypes
    bf16 = ml_dtypes.bfloat16
    in_maps = []
    for c in range(NCORES):
        b, g = c // 2, c % 2
        sl = slice(DL * g, DL * g + DL)
        in_maps.append({
            "xt": np.ascontiguousarray(x[b].T).astype(bf16),
            "wq": np.ascontiguousarray(w_qkv[:, sl]).astype(bf16),
            "wk": np.ascontiguousarray(w_qkv[:, EMBED:][:, sl]).astype(bf16),
            "wv": np.ascontiguousarray(w_qkv[:, 2 * EMBED:][:, sl]).astype(bf16),
            "bqs": np.ascontiguousarray(b_qkv[sl]).astype(np.float32),
            "bk": np.ascontiguousarray(b_qkv[EMBED:][sl]).astype(np.float32),
            "wp": np.ascontiguousarray(w_proj[sl, :]).astype(np.float16),
            "ident": np.eye(128, dtype=np.float16),
        })
    return in_maps


def kernel(x, w_qkv, b_qkv, w_proj, b_proj, _profile=False, _repeat=1):
    from concourse.bass_utils import run_bass_kernel_spmd

    x = np.asarray(x, dtype=np.float32)
    w_qkv = np.asarray(w_qkv, dtype=np.float32)
    b_qkv = np.asarray(b_qkv, dtype=np.float32)
    w_proj = np.asarray(w_proj, dtype=np.float32)
    b_proj = np.asarray(b_proj, dtype=np.float32)

    if _repeat not in _prog_cache:
        _prog_cache[_repeat] = _build_program(_repeat)
    nc = _prog_cache[_repeat]

    in_maps = _shard_inputs(x, w_qkv, b_qkv, w_proj)
    res = run_bass_kernel_spmd(
        nc, in_maps, list(range(NCORES)), trace=_profile,
    )

    # host-side gather: sum the two head-group partials per batch and add
    # the bias row (v-bias folded through w_proj, plus b_proj itself)
    bias_row = b_qkv[2 * EMBED:] @ w_proj + b_proj
    y = np.empty((B, T, EMBED), dtype=np.float32)
    for b in range(B):
        y[b] = (np.asarray(res.results[2 * b]["y"], dtype=np.float32)
                + np.asarray(res.results[2 * b + 1]["y"], dtype=np.float32)
                + bias_row)
    if _profile:
        return y, res
    return y


# revision 58
# speedup vs baseline: 1.0050x; 1.0013x over previous
"""Multi-head attention (B=4, T=2048, D=768, H=12) on 8 NeuronCores.

Sharding: core c handles batch b = c//2 and head-group g = c%2 (heads
6g..6g+5).  Each core computes its 6 heads' attention and a partial
output projection; the host sums the two partials per batch and adds
the bias terms (v-bias folds through w_proj since softmax rows sum to 1).

Device formulation (cost-model-shaped: every matmul streams with full
128-wide output partitions; PE is kept continuously busy so the p-state
stays at 2.4 GHz):

  qT/kT [384, 2048] = W.T @ xT  (bf16 inputs, f32 psum, f32r qT/kT)
  S^T [kpos 128, q] = kT_h.T @ qT_h    per (head, kpos-tile)
  P^T = exp(S^T)  -- ACT engine, or offloaded to DVE via the
        exp-as-int-bitcast trick (P in fp16)
  O [q 128, 64] += P^T_chunk.T @ v_h   accumulated over kpos tiles
  den[q, 1]    += P^T_chunk.T @ ones
  O_sb = O * (1/den)  (DVE per-partition scalar), fp16
  oT = DMA-crossbar transpose of O_sb per (pair, q-tile)
  y = oT.T @ wp  partial, fp16, host adds partner core + bias row

Schedule: a flat software pipeline over 12 sweeps x 16 kpos-tiles
(ticks): scores(t+1) | exp(t) | AV(t-2), with QKV projections of later
head-pairs, the output projection, and V emission injected as PE filler
paced by deadlines.  A fraction of exp tiles per sweep runs on DVE so
ACT never gates PE.
"""

import numpy as np

EMBED = 768
HEADS = 12
HD = 64
SCALE = HD ** -0.5
B, T = 4, 2048
NCORES = 8
HPC = 6            # heads per core
DL = HPC * HD      # 384 local model dims per core

NDT = EMBED // 128   # 6 contraction tiles over embed dim
NKT = T // 128       # 16 key-position tiles
NQT = T // 128       # 16 query row tiles
CW = 256             # qkv/proj chunk width
NCH = T // CW        # 8 chunks per pair row-block

# sweep order: qh-major inside pairs so each pair's first q-half finishes
# early enough for transposes/projection to overlap the next sweeps
SWEEPS = [(0, 0), (1, 0), (0, 1), (1, 1),
          (2, 0), (3, 0), (2, 1), (3, 1),
          (4, 0), (5, 0), (4, 1), (5, 1)]

# kt indices whose exp runs on DVE (bit-trick) instead of ACT, per sweep
OFFLOAD = {si: (2, 5, 8, 11, 14) for si in range(1, 7)}
for si in (7, 8, 9):
    OFFLOAD[si] = (2, 5, 8, 11, 15)
OFFLOAD[10] = (2, 6, 10, 15)
OFFLOAD[11] = (2, 6, 10, 14)
OFFLOAD[0] = ()

# exp-as-fp16-bits: bits = trunc(A*s + B); bitcast int16 -> fp16
EXP_A = float(np.float32(1024.0 / np.log(2.0)))
EXP_B = float(np.float32(15 * 1024 - 58.7))

_prog_cache = {}
DEBUG_DUMP = False


def _build_program(repeat=1):
    import concourse.bass as bass
    import concourse.mybir as mybir
    import concourse.tile as tile
    from concourse import bacc

    f32 = mybir.dt.float32
    f32r = mybir.dt.float32r
    fp16 = mybir.dt.float16
    bf16 = mybir.dt.bfloat16
    i16 = mybir.dt.int16
    ACT_EXP = mybir.ActivationFunctionType.Exp
    ADD = mybir.AluOpType.add
    MULT = mybir.AluOpType.mult

    nc = bacc.Bacc()

    xt_d = nc.dram_tensor("xt", [EMBED, T], bf16, kind="ExternalInput")
    wq_d = nc.dram_tensor("wq", [EMBED, DL], bf16, kind="ExternalInput")
    wk_d = nc.dram_tensor("wk", [EMBED, DL], bf16, kind="ExternalInput")
    wv_d = nc.dram_tensor("wv", [EMBED, DL], bf16, kind="ExternalInput")
    bqs_d = nc.dram_tensor("bqs", [DL], f32, kind="ExternalInput")
    bk_d = nc.dram_tensor("bk", [DL], f32, kind="ExternalInput")
    ident_d = nc.dram_tensor("ident", [128, 128], fp16, kind="ExternalInput")
    wp_d = nc.dram_tensor("wp", [DL, EMBED], fp16, kind="ExternalInput")
    y_d = nc.dram_tensor("y", [T, EMBED], fp16, kind="ExternalOutput")

    with tile.TileContext(nc) as tc:
      for _rep in range(repeat):
        with tc.tile_pool(name="persist", bufs=1) as pers, \
             tc.tile_pool(name="qk", bufs=2) as qk_pool, \
             tc.tile_pool(name="pT", bufs=8) as pT_pool, \
             tc.tile_pool(name="pss", bufs=2, space="PSUM") as pss_pool, \
             tc.tile_pool(name="po", bufs=2, space="PSUM") as po_pool, \
             tc.tile_pool(name="pfix", bufs=1, space="PSUM") as pfix_pool:

            # ---- persistent SBUF ----
            xt_sb = pers.tile([128, NDT, T], bf16, name="xt_sb")
            wq_sb = pers.tile([128, NDT, DL], bf16, name="wq_sb")
            wk_sb = pers.tile([128, NDT, DL], bf16, name="wk_sb")
            wv_sb = pers.tile([128, NDT, DL], bf16, name="wv_sb")
            wp_sb = pers.tile([128, 3, EMBED], fp16, name="wp_sb")
            v_sb = pers.tile([128, NKT, HPC, HD], fp16, name="v_sb")
            O_sb = pers.tile([128, NQT, DL], fp16, name="O_sb")
            oT_sb = pers.tile([128, 3, T], fp16, name="oT_sb")
            y01_sb = pers.tile([128, NQT, EMBED], fp16, name="y01_sb")
            rcp_sb = pers.tile([128, 32], f32, name="rcp_sb")
            bqs_sb = pers.tile([128, 3], f32, name="bqs_sb")
            bk_sb = pers.tile([128, 3], f32, name="bk_sb")
            ones_sb = pers.tile([128, 1], fp16, name="ones_sb")
            ident_sb = pers.tile([128, 128], fp16, name="ident_sb")
            warm_sb = pers.tile([128, 512], f32r, name="warm_sb")

            # ---- persistent PSUM (1 bank each) ----
            den_ps = pfix_pool.tile([128, 32], f32, name="den_ps")
            fill_ps = pfix_pool.tile([128, 512], f32, name="fill_ps")

            nc.vector.memset(ones_sb, 1.0)
            nc.vector.memset(warm_sb.bitcast(f32), 0.0)

            # ---- input DMAs (transfer-serialized; order = priority) ----
            def xt_dma(c):
                nc.sync.dma_start(out=xt_sb[:, :, bass.ts(c, 512)],
                                  in_=xt_d.ap()[:, bass.ts(c, 512)].rearrange("(n p) m -> p n m", p=128))
            # HWDGE (sync) for everything: the DMA device serializes all
            # transfers anyway and HWDGE setup is far cheaper than SWDGE
            # descriptor generation for these many-descriptor patterns.
            # First quarter-chunk of xt + wq first so q-chunk 0 can start
            # as early as possible.
            nc.sync.dma_start(out=xt_sb[:, :, 0:256],
                              in_=xt_d.ap()[:, 0:256].rearrange("(n p) m -> p n m", p=128))
            nc.sync.dma_start(out=wq_sb, in_=wq_d.ap().rearrange("(n p) m -> p n m", p=128))
            nc.sync.dma_start(out=xt_sb[:, :, 256:512],
                              in_=xt_d.ap()[:, 256:512].rearrange("(n p) m -> p n m", p=128))
            nc.sync.dma_start(out=wk_sb, in_=wk_d.ap().rearrange("(n p) m -> p n m", p=128))
            nc.gpsimd.dma_start(out=bqs_sb, in_=bqs_d.ap().rearrange("(n p) -> p n", p=128))
            nc.gpsimd.dma_start(out=bk_sb, in_=bk_d.ap().rearrange("(n p) -> p n", p=128))
            nc.sync.dma_start(out=wv_sb, in_=wv_d.ap().rearrange("(n p) m -> p n m", p=128))
            xt_dma(1)
            xt_dma(2)
            xt_dma(3)
            nc.gpsimd.dma_start(out=ident_sb, in_=ident_d.ap())
            nc.sync.dma_start(out=wp_sb, in_=wp_d.ap().rearrange("(n p) m -> p n m", p=128))

            # ---- helpers ----
            qk_tiles = {}

            def warm(n):
                for _ in range(n):
                    psw = pss_pool.tile([128, 1024], f32, name="psw", tag="pss")
                    nc.tensor.matmul(psw[:, 0:512], warm_sb[0:2, 0:128],
                                     warm_sb[0:2, :], start=True, stop=True)

            def alloc_pair(p):
                qTp = qk_pool.tile([128, T], f32r, name="qTp", tag="qT")
                kTp = qk_pool.tile([128, T], f32r, name="kTp", tag="kT")
                qk_tiles[p] = (qTp, kTp)

            def qk_half(p, which, ch, half):
                # half-chunk of the q or k projection for pair p
                qTp, kTp = qk_tiles[p]
                w_sb = wq_sb if which == "q" else wk_sb
                reg = fill_ps[:, 0:CW] if which == "q" else fill_ps[:, CW:2 * CW]
                csl = bass.ds(ch * CW, CW)
                dts = range(0, 3) if half == 0 else range(3, NDT)
                for dt in dts:
                    nc.tensor.matmul(reg, w_sb[:, dt, bass.ts(p, 128)],
                                     xt_sb[:, dt, csl],
                                     start=(dt == 0), stop=(dt == NDT - 1))
                if half == 1:
                    if which == "q":
                        nc.vector.tensor_scalar(
                            out=qTp[:, csl], in0=reg,
                            scalar1=bqs_sb[:, p:p + 1], scalar2=float(SCALE),
                            op0=ADD, op1=MULT)
                    else:
                        nc.vector.tensor_scalar_add(
                            out=kTp[:, csl], in0=reg, scalar1=bk_sb[:, p:p + 1])

            def emit_v(kt):
                psv = fill_ps[:, 0:DL]
                for dt in range(NDT):
                    nc.tensor.matmul(psv, xt_sb[:, dt, bass.ts(kt, 128)],
                                     wv_sb[:, dt, :],
                                     start=(dt == 0), stop=(dt == NDT - 1))

            def vcopy(kt):
                nc.vector.tensor_copy(
                    out=v_sb[:, kt],
                    in_=fill_ps[:, 0:DL].rearrange("p (h d) -> p h d", h=HPC))

            pss_of = {}   # tick -> pss tile
            pT_of = {}    # tick -> pT tile
            Ot_of = {}    # sweep -> O psum tile

            def scores(i, tk):
                si, h, qh, kt = tk
                hp, off = h // 2, (h % 2) * 64
                qTp, kTp = qk_tiles[hp]
                pss = pss_pool.tile([128, 1024], f32, name="pss", tag="pss")
                pss_of[i] = pss
                for c2 in range(2):
                    nc.tensor.matmul(
                        pss[:, bass.ts(c2, 512)],
                        kTp[off:off + 64, bass.ts(kt, 128)],
                        qTp[off:off + 64, bass.ds(qh * 1024 + c2 * 512, 512)],
                        start=True, stop=True)

            def exp_tile(i, tk):
                si, h, qh, kt = tk
                pss = pss_of.pop(i)
                pT = pT_pool.tile([128, 1024], fp16, name="pT", tag="pT")
                pT_of[i] = pT
                if kt in OFFLOAD[si]:
                    nc.vector.tensor_scalar(
                        out=pT.bitcast(i16), in0=pss,
                        scalar1=EXP_A, scalar2=EXP_B, op0=MULT, op1=ADD)
                else:
                    nc.scalar.activation(out=pT, in_=pss, func=ACT_EXP)

            def av(i, tk):
                si, h, qh, kt = tk
                if si not in Ot_of:
                    Ot_of[si] = po_pool.tile([128, 8, HD], f32, name="O_t", tag="po")
                O_t = Ot_of[si]
                pT = pT_of.pop(i)
                dcol = (si % 4) * 8
                # one accumulation group per PSUM bank per sweep: start only
                # on the very first matmul touching the bank (pends the whole
                # 2KB zero region; first write to each sub-range zeroes it),
                # stop on the last
                for ql in range(8):
                    nc.tensor.matmul(O_t[:, ql, :], pT[:, bass.ts(ql, 128)],
                                     v_sb[:, kt, h, :],
                                     start=(kt == 0 and ql == 0),
                                     stop=(kt == NKT - 1 and ql == 7),
                                     skip_group_check=True)
                for ql in range(8):
                    nc.tensor.matmul(den_ps[:, dcol + ql:dcol + ql + 1],
                                     pT[:, bass.ts(ql, 128)], ones_sb[:, 0:1],
                                     start=(kt == 0 and ql == 0),
                                     stop=(kt == NKT - 1 and ql == 7),
                                     skip_group_check=True)

            # ---- deferred engine work queues ----
            dve_hookq = []   # normalize items, <=3 popped per tick
            dma_hookq = []   # transpose items, <=2 popped per tick

            # pair-half transpose triggers: sweep-idx -> (pair, qt range)
            TRANSP = {1: (0, range(0, 8)), 3: (0, range(8, 16)),
                      5: (1, range(0, 8)), 7: (1, range(8, 16)),
                      9: (2, range(0, 8)), 11: (2, range(8, 16))}

            def push_normalize(si):
                h, qh = SWEEPS[si]
                O_t = Ot_of.pop(si)
                dcol = (si % 4) * 8
                off = h * HD

                def rcp():
                    nc.vector.reciprocal(out=rcp_sb[:, dcol:dcol + 8],
                                         in_=den_ps[:, dcol:dcol + 8])
                dve_hookq.append(rcp)
                for ql in range(8):
                    qt = qh * 8 + ql

                    def mul(ql=ql, qt=qt):
                        nc.vector.tensor_scalar_mul(
                            out=O_sb[:, qt, off:off + HD], in0=O_t[:, ql, :],
                            scalar1=rcp_sb[:, dcol + ql:dcol + ql + 1])
                    dve_hookq.append(mul)
                if si in TRANSP:
                    pair, qts = TRANSP[si]

                    def push_tr(pair=pair, qts=qts):
                        for qt in qts:
                            def tr(pair=pair, qt=qt):
                                nc.sync.dma_start_transpose(
                                    out=oT_sb[:, pair, bass.ts(qt, 128)],
                                    in_=O_sb[:, qt, bass.ds(128 * pair, 128)])
                            dma_hookq.append(tr)
                    dve_hookq.append(push_tr)

            # ---- filler units ----
            ydma_done = set()

            psy_ctr = [0]

            def proj_unit(qt, nh, dts, copy="dve"):
                # psy ping-pong halves of fill_ps; copy: "dve" | "act" | None
                # (None = dead PE-filler unit: result abandoned in psum)
                r = psy_ctr[0] % 2
                psy_ctr[0] += 1
                reg = fill_ps[:, r * CW:r * CW + CW]
                for j, dtp in enumerate(dts):
                    nc.tensor.matmul(reg, oT_sb[:, dtp, bass.ts(qt, 128)],
                                     wp_sb[:, dtp, bass.ds(nh * CW, CW)],
                                     start=(j == 0), stop=(j == len(dts) - 1))
                if copy == "dve":
                    nc.vector.tensor_copy(
                        out=y01_sb[:, qt, bass.ds(nh * CW, CW)], in_=reg)
                elif copy == "act":
                    nc.scalar.copy(
                        out=y01_sb[:, qt, bass.ds(nh * CW, CW)], in_=reg)
                if copy is not None and nh == 2:
                    nc.sync.dma_start(
                        out=y_d.ap()[bass.ds(128 * qt, 128), :],
                        in_=y01_sb[:, qt, :])
                    ydma_done.add(qt)

            # build filler list: (avail, deadline, cost, fn, dead)
            # consumed strictly in order; deadline-forced when due.
            # NOTE: units that open psum accumulation groups in the shared
            # fill bank must not interleave with other fill-bank groups —
            # ordered consumption + adjacent halves guarantees that.
            def build_fillers():
                units = []
                # pair-0 chunks 4-7, JIT inside sweep 0 (full chunks so no
                # other fill-bank group can open between their halves)
                for ch in range(4, NCH):
                    units.append((0, 2 * ch - 2, 640,
                                  (lambda ch=ch: (qk_half(0, "k", ch, 0),
                                                  qk_half(0, "k", ch, 1))),
                                  False))
                for ch in range(4, NCH):
                    units.append((0, 28, 640,
                                  (lambda ch=ch: (qk_half(0, "q", ch, 0),
                                                  qk_half(0, "q", ch, 1))),
                                  False))
                for p, base in ((1, 64), (2, 128)):
                    avail = base - 48
                    for ch in range(NCH):
                        qdl = (base - 2) if ch < 4 else (base + 29)
                        kdl = (base - 2) if ch < 4 else (base + 2 * ch - 4)
                        for half in (0, 1):
                            units.append((avail, qdl, 320,
                                          (lambda p=p, ch=ch, half=half:
                                           qk_half(p, "q", ch, half)), False))
                        for half in (0, 1):
                            units.append((avail, kdl, 320,
                                          (lambda p=p, ch=ch, half=half:
                                           qk_half(p, "k", ch, half)), False))
                # dead PE-filler: dt0+dt1 partial projections, results
                # abandoned (the real 3-dt projection happens later);
                # qt0-7 oT available after sweep idx5's transposes,
                # qt8-15 after idx7's
                for qt in range(0, 8):
                    for nh in range(3):
                        units.append((104, 10 ** 9, 220,
                                      (lambda qt=qt, nh=nh:
                                       proj_unit(qt, nh, (0, 1), None)), True))
                for qt in range(8, 16):
                    for nh in range(3):
                        units.append((138, 10 ** 9, 220,
                                      (lambda qt=qt, nh=nh:
                                       proj_unit(qt, nh, (0, 1), None)), True))
                # full projection for qt 0-7 (pair-2 qh0 oT ready ~tick 166)
                for qt in range(0, 8):
                    for nh in range(3):
                        units.append((167, 10 ** 9, 700,
                                      (lambda qt=qt, nh=nh:
                                       proj_unit(qt, nh, (0, 1, 2), "dve")),
                                      False))
                return units

            # ================= program =================
            alloc_pair(0)
            warm(10)
            for ch in (0, 1):
                qk_half(0, "q", ch, 0)
                qk_half(0, "q", ch, 1)
            warm(1)
            for ch in (0, 1):
                qk_half(0, "k", ch, 0)
                qk_half(0, "k", ch, 1)
            # V for kpos tiles 0-3 here: real PE work in the window where
            # the later xt/weight DMAs would otherwise leave PE starved
            emit_v(0)
            vcopy(0)
            emit_v(1)
            vcopy(1)
            for ch in (2, 3):
                qk_half(0, "q", ch, 0)
                qk_half(0, "q", ch, 1)
            emit_v(2)
            vcopy(2)
            for ch in (2, 3):
                qk_half(0, "k", ch, 0)
                qk_half(0, "k", ch, 1)
            emit_v(3)
            vcopy(3)

            ticks = [(si, h, qh, kt)
                     for si, (h, qh) in enumerate(SWEEPS) for kt in range(NKT)]
            units = build_fillers()
            alloc_ticks = {1: 16, 2: 80}
            uidx = 0
            spent = 0.0
            total_cost = sum(u[2] for u in units)
            BUDGET = total_cost / len(ticks)

            # prologue of the pipeline
            scores(0, ticks[0])
            exp_tile(0, ticks[0])

            if DEBUG_DUMP:
                dbg_p0_sb = pers.tile([128, 1024], fp16, name="dbg_p0_sb")
                nc.vector.tensor_copy(out=dbg_p0_sb, in_=pT_of[0])

            for i, tk in enumerate(ticks):
                si, h, qh, kt = tk
                for p, at in alloc_ticks.items():
                    if at == i and p not in qk_tiles:
                        alloc_pair(p)
                if i >= 2:
                    av(i - 2, ticks[i - 2])
                    psi = ticks[i - 2][0]
                    if ticks[i - 2][3] == NKT - 1:
                        push_normalize(psi)
                # deferred DVE / DMA hook items
                for _ in range(2):
                    if dve_hookq:
                        dve_hookq.pop(0)()
                for _ in range(3):
                    if dma_hookq:
                        dma_hookq.pop(0)()
                # paced fillers (ordered; deadline-forced when due)
                while uidx < len(units) and units[uidx][1] <= i:
                    spent += units[uidx][2]
                    units[uidx][3]()
                    uidx += 1
                while uidx < len(units) and units[uidx][0] <= i:
                    if units[uidx][4] and i >= 166:
                        uidx += 1   # drop stale dead-filler
                        continue
                    if spent >= (i + 1) * BUDGET:
                        break
                    spent += units[uidx][2]
                    units[uidx][3]()
                    uidx += 1
                # V emission four ticks ahead (kt 0-3 were done in the
                # prologue) so its DVE copy never gates a fill-bank group
                if si == 0 and kt + 4 < NKT:
                    emit_v(kt + 4)
                    vcopy(kt + 4)
                # scores + exp emitted together, one tick ahead: maximizes
                # the lead the ACT/DVE exp has over its AV consumer
                if i + 1 < len(ticks):
                    scores(i + 1, ticks[i + 1])
                    exp_tile(i + 1, ticks[i + 1])

            # ---- tail ----
            n = len(ticks)
            av(n - 2, ticks[n - 2])
            av(n - 1, ticks[n - 1])
            # leftover (non-dead) fillers first: they use the fill bank,
            # which the tail transposes then reuse
            while uidx < len(units):
                if not units[uidx][4]:
                    units[uidx][3]()
                uidx += 1
            while dve_hookq:
                dve_hookq.pop(0)()
            while dma_hookq:
                dma_hookq.pop(0)()
            # last sweep (5,1): normalize + PE-transpose + project, chained
            # per q-tile to minimize the drain. Transposes go through the PE
            # (identity matmul) instead of DMA: ~53ns each and no 3us DMA
            # round-trip on the critical path.
            O_t11 = Ot_of.pop(11)
            dcol = (11 % 4) * 8
            nc.vector.reciprocal(out=rcp_sb[:, dcol:dcol + 8],
                                 in_=den_ps[:, dcol:dcol + 8])
            trp_t = po_pool.tile([128, 8, HD], f32, name="trp", tag="po")
            tail_ps = [pss_pool.tile([128, 1024], f32, name="tailps", tag="pss")
                       for _ in range(2)]
            j = 0
            for ql in range(8):
                qt = 8 + ql
                nc.vector.tensor_scalar_mul(
                    out=O_sb[:, qt, 320:384], in0=O_t11[:, ql, :],
                    scalar1=rcp_sb[:, dcol + ql:dcol + ql + 1])
                tr_out = trp_t[:, ql, :].bitcast(fp16)
                nc.tensor.transpose(tr_out, O_sb[:, qt, 256:384], ident_sb)
                if ql % 2:
                    nc.vector.tensor_copy(out=oT_sb[:, 2, bass.ts(qt, 128)],
                                          in_=tr_out)
                else:
                    nc.scalar.copy(out=oT_sb[:, 2, bass.ts(qt, 128)],
                                   in_=tr_out)
                for nh in range(3):
                    reg = tail_ps[(j // 4) % 2][:, bass.ds((j % 4) * CW, CW)]
                    j += 1
                    for dtp in (0, 1, 2):
                        nc.tensor.matmul(reg, oT_sb[:, dtp, bass.ts(qt, 128)],
                                         wp_sb[:, dtp, bass.ds(nh * CW, CW)],
                                         start=(dtp == 0), stop=(dtp == 2),
                                         skip_group_check=True)
                    dst = y01_sb[:, qt, bass.ds(nh * CW, CW)]
                    if j % 2:
                        nc.vector.tensor_copy(out=dst, in_=reg)
                    else:
                        nc.scalar.copy(out=dst, in_=reg)
                    if ql == 7 and nh == 1:
                        # last q-tile: ship the first two thirds early so the
                        # final drain only waits on a short transfer
                        nc.sync.dma_start(out=y_d.ap()[bass.ds(128 * qt, 128), 0:512],
                                          in_=y01_sb[:, qt, 0:512])
                if ql == 7:
                    nc.sync.dma_start(out=y_d.ap()[bass.ds(128 * qt, 128), 512:768],
                                      in_=y01_sb[:, qt, 512:768])
                else:
                    nc.sync.dma_start(out=y_d.ap()[bass.ds(128 * qt, 128), :],
                                      in_=y01_sb[:, qt, :])
                ydma_done.add(qt)
            for qt in range(16):
                if qt not in ydma_done:
                    nc.sync.dma_start(out=y_d.ap()[bass.ds(128 * qt, 128), :],
                                      in_=y01_sb[:, qt, :])
                    ydma_done.add(qt)

            if DEBUG_DUMP:
                den_d = nc.dram_tensor("d_den", [128, 32], f32, kind="ExternalOutput")
                den_cp = pers.tile([128, 32], f32, name="den_cp")
                nc.vector.tensor_copy(out=den_cp, in_=den_ps)
                nc.sync.dma_start(out=den_d.ap(), in_=den_cp)
                qT0, kT0 = qk_tiles[0]
                dumps = [("d_qT0", qT0, f32), ("d_kT0", kT0, f32),
                         ("d_v", v_sb, fp16), ("d_O", O_sb, fp16),
                         ("d_oT", oT_sb, fp16), ("d_rcp", rcp_sb, f32),
                         ("d_p0", dbg_p0_sb, fp16)]
                for nm, t, dt_ in dumps:
                    sh = [t.shape[0], int(np.prod(t.shape[1:]))]
                    dd = nc.dram_tensor(nm, sh, t.dtype, kind="ExternalOutput")
                    nc.sync.dma_start(out=dd.ap(), in_=t.rearrange(
                        "p a b -> p (a b)") if len(t.shape) == 3 else (
                        t.rearrange("p a b c -> p (a b c)") if len(t.shape) == 4 else t))

    nc.finalize()
    return nc


def _shard_inputs(x, w_qkv, b_qkv, w_proj):
    import ml_dtypes
    bf16 = ml_dtypes.bfloat16
    in_maps = []
    for c in range(NCORES):
        b, g = c // 2, c % 2
        sl = slice(DL * g, DL * g + DL)
        in_maps.append({
            "xt": np.ascontiguousarray(x[b].T).astype(bf16),
            "wq": np.ascontiguousarray(w_qkv[:, sl]).astype(bf16),
            "wk": np.ascontiguousarray(w_qkv[:, EMBED:][:, sl]).astype(bf16),
            "wv": np.ascontiguousarray(w_qkv[:, 2 * EMBED:][:, sl]).astype(bf16),
            "bqs": np.ascontiguousarray(b_qkv[sl]).astype(np.float32),
            "bk": np.ascontiguousarray(b_qkv[EMBED:][sl]).astype(np.float32),
            "wp": np.ascontiguousarray(w_proj[sl, :]).astype(np.float16),
            "ident": np.eye(128, dtype=np.float16),
        })
    return in_maps


def kernel(x, w_qkv, b_qkv, w_proj, b_proj, _profile=False, _repeat=1):
    from concourse.bass_utils import run_bass_kernel_spmd

    x = np.asarray(x, dtype=np.float32)
    w_qkv = np.asarray(w_qkv, dtype=np.float32)
    b_qkv = np.asarray(b_qkv, dtype=np.float32)
    w_proj = np.asarray(w_proj, dtype=np.float32)
    b_proj = np.asarray(b_proj, dtype=np.float32)

    if _repeat not in _prog_cache:
        _prog_cache[_repeat] = _build_program(_repeat)
    nc = _prog_cache[_repeat]

    in_maps = _shard_inputs(x, w_qkv, b_qkv, w_proj)
    res = run_bass_kernel_spmd(
        nc, in_maps, list(range(NCORES)), trace=_profile,
    )

    # host-side gather: sum the two head-group partials per batch and add
    # the bias row (v-bias folded through w_proj, plus b_proj itself)
    bias_row = b_qkv[2 * EMBED:] @ w_proj + b_proj
    y = np.empty((B, T, EMBED), dtype=np.float32)
    for b in range(B):
        y[b] = (np.asarray(res.results[2 * b]["y"], dtype=np.float32)
                + np.asarray(res.results[2 * b + 1]["y"], dtype=np.float32)
                + bias_row)
    if _profile:
        return y, res
    return y
